# revision 1
# baseline (speedup 1.0000x reference)
"""IF spiking-neuron scan (charge / fire / hard-reset) on 8 Trainium2 cores.

Reference recurrence over t (elementwise on every [B, N] element):
    v = v + x_t
    s = (v - 1.0 >= 0)          # spike, 0.0/1.0
    v = (1 - s) * v             # hard reset to 0

Sharding: pure data parallel over the B*N = 262144 element dimension;
each of the 8 cores owns 32768 element chains with zero communication.
Per core the chains live in SBUF as a [128, 256] f32 state tile; the
64-step scan runs locally, bit-exact vs the reference.

v2 design (measured on HW):
  - The serial 64-step chain on the DVE is the wall: 2 ops/step
    (tensor_tensor add + fused scalar_tensor_tensor reset) in two
    interleaved half-width streams paces at ~850 ns/step; all other
    structures measured worse (Pool-engine offload, single full-width
    stream, cross-engine ping-pong).
  - Input x is pre-transposed on the host to [P, T, F] per core, so a
    timestep block is ONE contiguous multi-KiB descriptor per partition
    (the old [T, P*F] layout forced 1 KiB descriptors at ring rate).
    Every block is split across BOTH hardware DGE rings (SP + ACT,
    ~400 GB/s standalone, ~350 GB/s while the DVE streams), and all
    input dma_starts are hoisted before the compute so the in-order
    sequencers queue them up front; input never gates the chain.
  - Spikes are ONE ACT pass per block: z = Sign(V_TH - u) written
    directly as uint8. The float->uint8 conversion saturates negatives
    to 0 (verified on HW), so z = [u < V_TH] = 1 - s exactly, including
    u == V_TH (Sign(0) = 0 -> spike). The host flips it back. The last
    blocks compute z on the DVE instead (tensor_scalar is_lt) - the
    ACT handoff there is a pure drain tail.
  - The whole input stays resident in SBUF (x pool never recycles);
    the u history is ONE flat [P, T*F] buffer - each step writes its
    own disjoint range, so there is no WAR edge at all and the
    block-boundary waits fuse into the chain instructions (paired
    A/B-measured -454 ns vs a 6-deep cycling pool).
  - Small blocks at both ends shrink pipeline fill (first timestep is
    one flank per ring; u_0 = x_0 skips the first add) and drain (the
    final 1-step blocks keep the last z + output DMA tiny). The first
    6 timesteps are per-step column-split across the rings (paired
    A/B-measured -550 ns: each half-stream's data lands one ring
    transfer earlier, closing the ~0.6 us of head gaps in the chain).
"""

import numpy as np

import concourse.tile as tile
from concourse import bacc, mybir
from concourse.bass_utils import run_bass_kernel_spmd

T = 64
B = 32
N = 8192
NCORES = 8
PERCORE = (B * N) // NCORES  # 32768 element chains per core
P = 128                      # SBUF partitions
F = PERCORE // P             # 256 elements per partition
H = F // 2                   # half-width for the two interleaved streams

V_TH = 1.0

# timestep block sizes: small at the edges to cut pipeline fill/drain
# (block 0 is a single step: u_0 = x_0, so the chain starts the moment
# one timestep lands; the 1-step final blocks shrink the ACT+DMA drain)
BLOCKS = [1, 2, 3, 4, 6] + [8] * 5 + [4, 2, 1, 1]
assert sum(BLOCKS) == T

_NC_CACHE = {}


def build_nc(blocks=None, fine_head_steps=6, quarter_step0=False,
             single_dma_after=None, flat_ub=True, flat_st=True, flat_x=True):
    blocks = list(BLOCKS if blocks is None else blocks)
    # Bacc (not raw Bass): its compile() splits multi-wait sync conditions
    # into nop/event-semaphore prefixes — walrus accepts at most one sync
    # wait per hardware instruction.
    nc = bacc.Bacc("TRN2", target_bir_lowering=False, debug=False)
    x = nc.dram_tensor("x", [P, T, F], mybir.dt.float32, kind="ExternalInput").ap()
    y = nc.dram_tensor("y", [P, T, F], mybir.dt.uint8, kind="ExternalOutput").ap()

    with tile.TileContext(nc) as tc:
        with (
            tc.tile_pool(name="xin", bufs=len(blocks)) as xpool,
            tc.tile_pool(name="sout", bufs=4) as spool,
            tc.tile_pool(name="ub", bufs=6) as ubpool,
            tc.tile_pool(name="ubf", bufs=1) as ubfpool,
            tc.tile_pool(name="stf", bufs=1) as stfpool,
            tc.tile_pool(name="xf", bufs=1) as xfpool,
            tc.tile_pool(name="v", bufs=1) as vpool,
        ):
            v = vpool.tile([P, F], mybir.dt.float32)
            nc.vector.memset(v[:], 0.0)
            # Issue ALL input dma_starts first: input tiles have no deps, so
            # both DGE rings fill their descriptor queues up front and stream
            # ahead of the chain. (Interleaving them with the per-block ACT
            # ops would serialize each ring's next input chunk behind the
            # previous block's activation - the sequencers are in-order.)
            xflat = None
            if flat_x:
                xflat = xfpool.tile([P, T * F], mybir.dt.float32)
            xts = []
            t0 = 0
            for bi, tb in enumerate(blocks):
                if flat_x:
                    xt = xflat[:, t0 * F:(t0 + tb) * F]
                else:
                    xt = xpool.tile([P, tb * F], mybir.dt.float32, tag="xin")
                # split EVERY block across both HW-DGE rings: each block
                # lands in half the time and the early blocks never gate
                # the chain (ring rate drops to ~170 GB/s once the DVE is
                # streaming, so a single ring can fall behind at the start)
                if t0 == 0 and tb == 1 and quarter_step0:
                    q = F // 4
                    for qi in range(4):
                        eng = nc.sync if qi % 2 == 0 else nc.scalar
                        eng.dma_start(xt[:, qi * q:(qi + 1) * q],
                                      x[:, 0, qi * q:(qi + 1) * q])
                elif single_dma_after is not None and t0 >= single_dma_after:
                    # slack-rich region: ONE writer per x tile, so the
                    # block's chain wait fuses into its first instruction
                    # (dual-flank tiles force a standalone sem-wait prefix
                    # on the DVE queue); rings alternate per block
                    eng = nc.sync if bi % 2 == 0 else nc.scalar
                    eng.dma_start(xt[:], x[:, t0:t0 + tb, :])
                elif tb == 1 or t0 + tb <= fine_head_steps:
                    # per-step column-split: each half-stream of the chain
                    # (and each ACT/z half) waits only on its own ring's
                    # flank, and each step lands a ring-transfer earlier
                    for ti in range(tb):
                        lo = ti * F
                        nc.sync.dma_start(xt[:, lo:lo + H], x[:, t0 + ti, :H])
                        nc.scalar.dma_start(xt[:, lo + H:lo + F], x[:, t0 + ti, H:])
                else:
                    th = (tb + 1) // 2
                    nc.sync.dma_start(xt[:, :th * F], x[:, t0:t0 + th, :])
                    nc.scalar.dma_start(xt[:, th * F:], x[:, t0 + th:t0 + tb, :])
                xts.append(xt)
                t0 += tb
            stflat = None
            if flat_st:
                # flat spike buffer: disjoint per-block ranges, so the
                # tail z ops on the DVE never wait on an output DMA (WAR)
                stflat = stfpool.tile([P, T * F], mybir.dt.uint8)
            ubflat = None
            if flat_ub:
                # one flat u-history buffer: every step writes its own
                # disjoint range, so there is NO ub WAR edge and the
                # block-boundary waits on the DVE queue fuse away
                ubflat = ubfpool.tile([P, T * F], mybir.dt.float32)
            t0 = 0
            for bi, tb in enumerate(blocks):
                xt = xts[bi]
                if flat_ub:
                    ub = ubflat[:, t0 * F:(t0 + tb) * F]
                else:
                    ub = ubpool.tile([P, tb * F], mybir.dt.float32, tag="ub")
                for ti in range(tb):
                    if t0 + ti == 0:
                        # v_0 = 0, so u_0 = x_0: skip the add, the reset and
                        # the spike pass read the x tile directly
                        nparts = 4 if quarter_step0 else 2
                        w = F // nparts
                        for h in range(nparts):
                            nc.vector.scalar_tensor_tensor(
                                v[:, h * w:(h + 1) * w], xt[:, h * w:h * w + w],
                                V_TH, xt[:, h * w:h * w + w],
                                mybir.AluOpType.is_lt, mybir.AluOpType.mult,
                            )
                        continue
                    for h in range(2):
                        lo = ti * F + h * H
                        nc.vector.tensor_add(
                            ub[:, lo:lo + H], v[:, h * H:(h + 1) * H],
                            xt[:, lo:lo + H],
                        )
                    if t0 + ti == T - 1:
                        continue  # v after the final timestep is never read
                    for h in range(2):
                        lo = ti * F + h * H
                        nc.vector.scalar_tensor_tensor(
                            v[:, h * H:(h + 1) * H], ub[:, lo:lo + H], V_TH,
                            ub[:, lo:lo + H],
                            mybir.AluOpType.is_lt, mybir.AluOpType.mult,
                        )
                if flat_st:
                    st = stflat[:, t0 * F:(t0 + tb) * F]
                else:
                    st = spool.tile([P, tb * F], mybir.dt.uint8, tag="sout")
                usrc = xt if t0 == 0 and tb == 1 else ub
                if t0 + tb > T - 3:
                    # final blocks: the ACT hop (engine handoff + 222-cycle
                    # SBUF latency) is a pure tail; one DVE op computes
                    # z = (u < V_TH) directly instead
                    nc.vector.tensor_scalar(
                        st[:], usrc[:], V_TH, None, mybir.AluOpType.is_lt
                    )
                else:
                    # One ACT pass: z = Sign(V_TH - u) in {-1,0,1}; the uint8
                    # store saturates to {0,1}, so z = [u < V_TH] = 1 - s
                    # exactly (u == V_TH -> Sign(0) = 0 -> spike). Host flips.
                    nc.scalar.activation(
                        st[:], usrc[:], mybir.ActivationFunctionType.Sign,
                        bias=V_TH, scale=-1.0,
                    )
                # outputs alternate rings; they enqueue behind that ring's
                # input chunks (FIFO), which are long done by then
                oeng = nc.sync if bi % 2 == 0 else nc.scalar
                oeng.dma_start(y[:, t0:t0 + tb, :], st[:])
                t0 += tb
    nc.compile()
    return nc


def _get_nc():
    if "nc" not in _NC_CACHE:
        _NC_CACHE["nc"] = build_nc()
    return _NC_CACHE["nc"]


def run_sharded(x_seq, trace=False, nc=None, **kwargs):
    if nc is None:
        nc = _get_nc()
    x2 = np.asarray(x_seq, dtype=np.float32).reshape(T, B * N)
    in_maps = []
    for c in range(NCORES):
        # core slab [T, PERCORE] -> [P, T, F]: partition-major, time
        # contiguous per partition so each block is one fat descriptor
        xc = x2[:, c * PERCORE:(c + 1) * PERCORE].reshape(T, P, F)
        in_maps.append({"x": np.ascontiguousarray(xc.transpose(1, 0, 2))})
    # A cold device occasionally reports NRT_EXEC_UNIT_UNRECOVERABLE on the
    # first execute and recovers on the next attempt; retry a couple times.
    for attempt in range(3):
        try:
            res = run_bass_kernel_spmd(
                nc, in_maps, list(range(NCORES)), trace=trace, **kwargs
            )
            break
        except Exception:  # jax.errors.JaxRuntimeError and friends
            if attempt == 2:
                raise
            import time
            time.sleep(2.0)
    out = np.empty((T, B * N), dtype=np.float32)
    for c in range(NCORES):
        zc = np.asarray(res.results[c]["y"])          # [P, T, F] uint8, z = 1-s
        r = zc.transpose(1, 0, 2).reshape(T, PERCORE)
        out[:, c * PERCORE:(c + 1) * PERCORE] = 1 - r
    return out.reshape(T, B, N), res


def kernel(x_seq):
    out, _ = run_sharded(x_seq)
    return out



# revision 2
# speedup vs baseline: 1.2650x; 1.2650x over previous
"""IF spiking-neuron scan (charge / fire / hard-reset) on 8 Trainium2 cores.

Reference recurrence over t (elementwise on every [B, N] element):
    v = v + x_t
    s = (v - 1.0 >= 0)          # spike, 0.0/1.0
    v = (1 - s) * v             # hard reset to 0

Sharding: pure data parallel over the B*N = 262144 element chains;
each of the 8 cores owns 32768 chains with zero communication.
Per core the chains live in SBUF as 128 partitions x 256 columns; the
64-step scan runs locally, bit-exact vs the reference.

v3 design (custom fused DVE op):
  - The recurrence is restated on the pre-reset potential u_t = v_t + x_t:
        u_t = select(u_{t-1} < 1, u_{t-1}, 0) + x_t
    and a runtime-registered custom DVE uop (IF_STEP_ANT) computes that
    in ONE Vector instruction per step per stream (the v2 kernel needed
    two: tensor_add + scalar_tensor_tensor).  The DVE chain was the wall
    at ~69 us busy; fusing halves the instruction count.
  - Spikes are ONE ACT pass per block: z = Sign(V_TH - u) written
    directly as uint8. The float->uint8 conversion saturates negatives
    to 0, so z = [u < V_TH] = 1 - s exactly, including u == V_TH
    (Sign(0) = 0 -> spike). The host flips it back. The last blocks
    compute z on the DVE instead (tensor_scalar is_lt) as a drain tail.
  - Input x is pre-transposed on the host to [P, T, F] per core, so a
    timestep block is ONE contiguous multi-KiB descriptor per partition.
    Every block is split across BOTH hardware DGE rings (SP + ACT), and
    all input dma_starts are hoisted before the compute so the in-order
    sequencers queue them up front; input never gates the chain.
  - The whole input stays resident in SBUF; the u history is ONE flat
    [P, T*F] buffer - each step writes its own disjoint range, so there
    is no WAR edge at all.
  - Small blocks at both ends shrink pipeline fill (first timestep is
    one flank per ring; u_0 = x_0 skips the first op) and drain.
"""

import numpy as np

import concourse.dve_ops as dve_ops
import concourse.tile as tile
from concourse import bacc, mybir
from concourse.bass_utils import run_bass_kernel_spmd
from concourse.dve_spec import Spec, Src0, Src1, C0, Zero, select, lower
from concourse.dve_uop import DveOpSpec

T = 64
B = 32
N = 8192
NCORES = 8
PERCORE = (B * N) // NCORES  # 32768 element chains per core
P = 128                      # SBUF partitions
F = PERCORE // P             # 256 elements per partition
H = F // 2                   # half-width for the two interleaved streams

V_TH = 1.0

# timestep block sizes: small at the edges to cut pipeline fill/drain
BLOCKS = [1, 2, 3, 4, 6] + [8] * 5 + [4, 2, 1, 1]
assert sum(BLOCKS) == T

_NC_CACHE = {}


def register_if_step():
    """Runtime-register the fused IF-step custom DVE op:
        out = select(in0 < s0, in0, 0) + in1
    i.e. hard-reset the carried potential where it crossed threshold,
    then charge with the new input -- the whole per-step recurrence in
    one Vector instruction. Uses the standard extension point
    (dve_ops.OPS + sub-opcode registry); the uop table is generated
    per-NEFF, so no firmware change is involved."""
    name = "IF_STEP_ANT"
    for op in dve_ops.OPS:
        if op.name == name:
            return op
    spec = Spec(
        body=select(Src0 < C0, Src0, Zero) + Src1,
        reference=lambda in0, in1, s0, s1, imm2: np.where(in0 < s0, in0, 0.0).astype(
            np.float32
        )
        + in1,
    )
    row = dve_ops._CUSTOM_DVE_ROW_BASE + len(dve_ops.OPS)
    shas = {}
    for ver in ("v3", "v4"):
        s = DveOpSpec(name=name, opcode=row, uops=lower(spec, ver=ver), rd1_en=True)
        shas[ver] = s.sha(ver)
    op = dve_ops.DveOp(name, spec, subdim=False, uops_sha=shas)
    dve_ops.OPS.append(op)
    dve_ops._SUB_OPCODE_FOR_NAME[name] = row
    dve_ops.CUSTOM_DVE_SPECS[name] = spec
    return op


IF_STEP = register_if_step()


def build_nc(blocks=None, fine_head_steps=6, nstreams=2, tail_dve_steps=3):
    blocks = list(BLOCKS if blocks is None else blocks)
    # Bacc (not raw Bass): its compile() splits multi-wait sync conditions
    # into nop/event-semaphore prefixes — walrus accepts at most one sync
    # wait per hardware instruction.
    nc = bacc.Bacc("TRN2", target_bir_lowering=False, debug=False)
    x = nc.dram_tensor("x", [P, T, F], mybir.dt.float32, kind="ExternalInput").ap()
    y = nc.dram_tensor("y", [P, T, F], mybir.dt.uint8, kind="ExternalOutput").ap()

    with tile.TileContext(nc) as tc:
        with (
            tc.tile_pool(name="xf", bufs=1) as xfpool,
            tc.tile_pool(name="ubf", bufs=1) as ubfpool,
            tc.tile_pool(name="stf", bufs=1) as stfpool,
        ):
            # Issue ALL input dma_starts first: input tiles have no deps, so
            # both DGE rings fill their descriptor queues up front and stream
            # ahead of the chain.
            xflat = xfpool.tile([P, T * F], mybir.dt.float32)
            t0 = 0
            for bi, tb in enumerate(blocks):
                xt = xflat[:, t0 * F:(t0 + tb) * F]
                if tb == 1 or t0 + tb <= fine_head_steps:
                    # per-step column-split: each half-stream of the chain
                    # waits only on its own ring's flank, and each step
                    # lands a ring-transfer earlier
                    for ti in range(tb):
                        lo = ti * F
                        nc.sync.dma_start(xt[:, lo:lo + H], x[:, t0 + ti, :H])
                        nc.scalar.dma_start(xt[:, lo + H:lo + F], x[:, t0 + ti, H:])
                else:
                    th = (tb + 1) // 2
                    nc.sync.dma_start(xt[:, :th * F], x[:, t0:t0 + th, :])
                    nc.scalar.dma_start(xt[:, th * F:], x[:, t0 + th:t0 + tb, :])
                t0 += tb

            # flat spike buffer: disjoint per-block ranges, so the tail z
            # ops on the DVE never wait on an output DMA (WAR)
            stflat = stfpool.tile([P, T * F], mybir.dt.uint8)
            # one flat u-history buffer: every step writes its own disjoint
            # range, so there is NO ub WAR edge at all
            ubflat = ubfpool.tile([P, T * F], mybir.dt.float32)

            sw = F // nstreams  # stream width
            t0 = 0
            for bi, tb in enumerate(blocks):
                for ti in range(tb):
                    t = t0 + ti
                    if t == 0:
                        # v_0 = 0, so u_0 = x_0: the spike pass and the
                        # t=1 chain op read the x tile directly
                        continue
                    src = xflat if t == 1 else ubflat
                    for h in range(nstreams):
                        lo = t * F + h * sw
                        plo = (t - 1) * F + h * sw
                        nc.vector._custom_dve(
                            IF_STEP,
                            out=ubflat[:, lo:lo + sw],
                            in0=src[:, plo:plo + sw],
                            in1=xflat[:, lo:lo + sw],
                            s0=V_TH,
                        )
                st = stflat[:, t0 * F:(t0 + tb) * F]
                usrc = xflat if t0 == 0 and tb == 1 else ubflat
                usl = usrc[:, t0 * F:(t0 + tb) * F]
                if t0 + tb > T - tail_dve_steps:
                    # final blocks: the ACT hop (engine handoff + 222-cycle
                    # SBUF latency) is a pure tail; one DVE op computes
                    # z = (u < V_TH) directly instead
                    nc.vector.tensor_scalar(
                        st[:], usl, V_TH, None, mybir.AluOpType.is_lt
                    )
                else:
                    # One ACT pass: z = Sign(V_TH - u) in {-1,0,1}; the uint8
                    # store saturates to {0,1}, so z = [u < V_TH] = 1 - s
                    # exactly (u == V_TH -> Sign(0) = 0 -> spike). Host flips.
                    nc.scalar.activation(
                        st[:], usl, mybir.ActivationFunctionType.Sign,
                        bias=V_TH, scale=-1.0,
                    )
                # outputs alternate rings; they enqueue behind that ring's
                # input chunks (FIFO), which are long done by then
                oeng = nc.sync if bi % 2 == 0 else nc.scalar
                oeng.dma_start(y[:, t0:t0 + tb, :], st[:])
                t0 += tb
    nc.compile()
    return nc


def _get_nc():
    if "nc" not in _NC_CACHE:
        _NC_CACHE["nc"] = build_nc()
    return _NC_CACHE["nc"]


def run_sharded(x_seq, trace=False, nc=None, **kwargs):
    if nc is None:
        nc = _get_nc()
    x2 = np.asarray(x_seq, dtype=np.float32).reshape(T, B * N)
    in_maps = []
    for c in range(NCORES):
        # core slab [T, PERCORE] -> [P, T, F]: partition-major, time
        # contiguous per partition so each block is one fat descriptor
        xc = x2[:, c * PERCORE:(c + 1) * PERCORE].reshape(T, P, F)
        in_maps.append({"x": np.ascontiguousarray(xc.transpose(1, 0, 2))})
    # A cold device occasionally reports NRT_EXEC_UNIT_UNRECOVERABLE on the
    # first execute and recovers on the next attempt; retry a couple times.
    for attempt in range(3):
        try:
            res = run_bass_kernel_spmd(
                nc, in_maps, list(range(NCORES)), trace=trace, **kwargs
            )
            break
        except Exception:  # jax.errors.JaxRuntimeError and friends
            if attempt == 2:
                raise
            import time
            time.sleep(2.0)
    out = np.empty((T, B * N), dtype=np.float32)
    for c in range(NCORES):
        zc = np.asarray(res.results[c]["y"])          # [P, T, F] uint8, z = 1-s
        r = zc.transpose(1, 0, 2).reshape(T, PERCORE)
        out[:, c * PERCORE:(c + 1) * PERCORE] = 1 - r
    return out.reshape(T, B, N), res


def kernel(x_seq):
    out, _ = run_sharded(x_seq)
    return out


# revision 4
# speedup vs baseline: 1.3587x; 1.0741x over previous
"""IF spiking-neuron scan (charge / fire / hard-reset) on 8 Trainium2 cores.

Reference recurrence over t (elementwise on every [B, N] element):
    v = v + x_t
    s = (v - 1.0 >= 0)          # spike, 0.0/1.0
    v = (1 - s) * v             # hard reset to 0

Sharding: pure data parallel over the B*N = 262144 element chains;
each of the 8 cores owns 32768 chains with zero communication.
Per core the chains live in SBUF as 128 partitions x 256 columns; the
64-step scan runs locally, bit-exact vs the reference.

v3 design (custom fused DVE op):
  - The recurrence is restated on the pre-reset potential u_t = v_t + x_t:
        u_t = select(u_{t-1} < 1, u_{t-1}, 0) + x_t
    and a runtime-registered custom DVE uop (IF_STEP_ANT) computes that
    in ONE Vector instruction per step per stream (the v2 kernel needed
    two: tensor_add + scalar_tensor_tensor).  The DVE chain was the wall
    at ~69 us busy; fusing halves the instruction count.
  - Spikes are ONE ACT pass per block: z = Sign(V_TH - u) written
    directly as uint8. The float->uint8 conversion saturates negatives
    to 0, so z = [u < V_TH] = 1 - s exactly, including u == V_TH
    (Sign(0) = 0 -> spike). The host flips it back. The last blocks
    compute z on the DVE instead (tensor_scalar is_lt) as a drain tail.
  - Input x is pre-transposed on the host to [P, T, F] per core, so a
    timestep block is ONE contiguous multi-KiB descriptor per partition.
    Every block is split across BOTH hardware DGE rings (SP + ACT), and
    all input dma_starts are hoisted before the compute so the in-order
    sequencers queue them up front; input never gates the chain.
  - The whole input stays resident in SBUF; the u history is ONE flat
    [P, T*F] buffer - each step writes its own disjoint range, so there
    is no WAR edge at all.
  - Small blocks at both ends shrink pipeline fill (first timestep is
    one flank per ring; u_0 = x_0 skips the first op) and drain.
"""

import numpy as np

import concourse.dve_ops as dve_ops
import concourse.tile as tile
from concourse import bacc, mybir
from concourse.bass_utils import run_bass_kernel_spmd
from concourse.dve_spec import Spec, Src0, Src1, C0, Zero, select, lower
from concourse.dve_uop import DveOpSpec

T = 64
B = 32
N = 8192
NCORES = 8
PERCORE = (B * N) // NCORES  # 32768 element chains per core
P = 128                      # SBUF partitions
F = PERCORE // P             # 256 elements per partition
H = F // 2                   # half-width for the two interleaved streams

V_TH = 1.0

# timestep block sizes: small at the edges to cut pipeline fill/drain
BLOCKS = [1, 2, 3, 4, 6] + [8] * 5 + [4, 2, 1, 1]
assert sum(BLOCKS) == T

_NC_CACHE = {}


def register_if_step():
    """Runtime-register the fused IF-step custom DVE op:
        out = select(in0 < s0, in0, 0) + in1
    i.e. hard-reset the carried potential where it crossed threshold,
    then charge with the new input -- the whole per-step recurrence in
    one Vector instruction. Uses the standard extension point
    (dve_ops.OPS + sub-opcode registry); the uop table is generated
    per-NEFF, so no firmware change is involved."""
    name = "IF_STEP_ANT"
    for op in dve_ops.OPS:
        if op.name == name:
            return op
    spec = Spec(
        body=select(Src0 < C0, Src0, Zero) + Src1,
        reference=lambda in0, in1, s0, s1, imm2: np.where(in0 < s0, in0, 0.0).astype(
            np.float32
        )
        + in1,
    )
    row = dve_ops._CUSTOM_DVE_ROW_BASE + len(dve_ops.OPS)
    shas = {}
    for ver in ("v3", "v4"):
        s = DveOpSpec(name=name, opcode=row, uops=lower(spec, ver=ver), rd1_en=True)
        shas[ver] = s.sha(ver)
    op = dve_ops.DveOp(name, spec, subdim=False, uops_sha=shas)
    dve_ops.OPS.append(op)
    dve_ops._SUB_OPCODE_FOR_NAME[name] = row
    dve_ops.CUSTOM_DVE_SPECS[name] = spec
    return op


IF_STEP = register_if_step()


def build_nc(blocks=None, fine_head_steps=6, nstreams=2, tail_dve_steps=3):
    blocks = list(BLOCKS if blocks is None else blocks)
    # Bacc (not raw Bass): its compile() splits multi-wait sync conditions
    # into nop/event-semaphore prefixes — walrus accepts at most one sync
    # wait per hardware instruction.
    nc = bacc.Bacc("TRN2", target_bir_lowering=False, debug=False)
    x = nc.dram_tensor("x", [P, T, F], mybir.dt.float32, kind="ExternalInput").ap()
    y = nc.dram_tensor("y", [P, T, F], mybir.dt.uint8, kind="ExternalOutput").ap()

    with tile.TileContext(nc) as tc:
        with (
            tc.tile_pool(name="xf", bufs=1) as xfpool,
            tc.tile_pool(name="ubf", bufs=1) as ubfpool,
            tc.tile_pool(name="stf", bufs=1) as stfpool,
        ):
            # Issue ALL input dma_starts first: input tiles have no deps, so
            # the input DGE rings fill their descriptor queues up front and
            # stream ahead of the chain. Inputs get TWO dedicated rings
            # (SP + GpSimd); outputs go on the ACT ring, so they stream out
            # during the chain instead of queueing behind 8 MiB of input.
            xflat = xfpool.tile([P, T * F], mybir.dt.float32)
            t0 = 0
            for bi, tb in enumerate(blocks):
                xt = xflat[:, t0 * F:(t0 + tb) * F]
                if tb == 1 or t0 + tb <= fine_head_steps:
                    # per-step column-split: each half-stream of the chain
                    # waits only on its own ring's flank, and each step
                    # lands a ring-transfer earlier
                    for ti in range(tb):
                        lo = ti * F
                        nc.sync.dma_start(xt[:, lo:lo + H], x[:, t0 + ti, :H])
                        nc.gpsimd.dma_start(xt[:, lo + H:lo + F], x[:, t0 + ti, H:])
                else:
                    th = (tb + 1) // 2
                    nc.sync.dma_start(xt[:, :th * F], x[:, t0:t0 + th, :])
                    nc.gpsimd.dma_start(xt[:, th * F:], x[:, t0 + th:t0 + tb, :])
                t0 += tb

            # flat spike buffer: disjoint per-block ranges, so the tail z
            # ops on the DVE never wait on an output DMA (WAR)
            stflat = stfpool.tile([P, T * F], mybir.dt.uint8)
            # one flat u-history buffer: every step writes its own disjoint
            # range, so there is NO ub WAR edge at all
            ubflat = ubfpool.tile([P, T * F], mybir.dt.float32)

            sw = F // nstreams  # stream width
            t0 = 0
            for bi, tb in enumerate(blocks):
                for ti in range(tb):
                    t = t0 + ti
                    if t == 0:
                        # v_0 = 0, so u_0 = x_0: the spike pass and the
                        # t=1 chain op read the x tile directly
                        continue
                    src = xflat if t == 1 else ubflat
                    for h in range(nstreams):
                        lo = t * F + h * sw
                        plo = (t - 1) * F + h * sw
                        nc.vector._custom_dve(
                            IF_STEP,
                            out=ubflat[:, lo:lo + sw],
                            in0=src[:, plo:plo + sw],
                            in1=xflat[:, lo:lo + sw],
                            s0=V_TH,
                        )
                st = stflat[:, t0 * F:(t0 + tb) * F]
                usrc = xflat if t0 == 0 and tb == 1 else ubflat
                usl = usrc[:, t0 * F:(t0 + tb) * F]
                if t0 + tb > T - tail_dve_steps:
                    # final blocks: the ACT hop (engine handoff + 222-cycle
                    # SBUF latency) is a pure tail; one DVE op computes
                    # z = (u < V_TH) directly instead
                    nc.vector.tensor_scalar(
                        st[:], usl, V_TH, None, mybir.AluOpType.is_lt
                    )
                else:
                    # One ACT pass: z = Sign(V_TH - u) in {-1,0,1}; the uint8
                    # store saturates to {0,1}, so z = [u < V_TH] = 1 - s
                    # exactly (u == V_TH -> Sign(0) = 0 -> spike). Host flips.
                    nc.scalar.activation(
                        st[:], usl, mybir.ActivationFunctionType.Sign,
                        bias=V_TH, scale=-1.0,
                    )
                # all outputs ride the ACT ring (no input ahead of them),
                # issued right after the block's z pass
                nc.scalar.dma_start(y[:, t0:t0 + tb, :], st[:])
                t0 += tb
    nc.compile()
    return nc


def _get_nc():
    if "nc" not in _NC_CACHE:
        _NC_CACHE["nc"] = build_nc()
    return _NC_CACHE["nc"]


def run_sharded(x_seq, trace=False, nc=None, **kwargs):
    if nc is None:
        nc = _get_nc()
    x2 = np.asarray(x_seq, dtype=np.float32).reshape(T, B * N)
    in_maps = []
    for c in range(NCORES):
        # core slab [T, PERCORE] -> [P, T, F]: partition-major, time
        # contiguous per partition so each block is one fat descriptor
        xc = x2[:, c * PERCORE:(c + 1) * PERCORE].reshape(T, P, F)
        in_maps.append({"x": np.ascontiguousarray(xc.transpose(1, 0, 2))})
    # A cold device occasionally reports NRT_EXEC_UNIT_UNRECOVERABLE on the
    # first execute and recovers on the next attempt; retry a couple times.
    for attempt in range(3):
        try:
            res = run_bass_kernel_spmd(
                nc, in_maps, list(range(NCORES)), trace=trace, **kwargs
            )
            break
        except Exception:  # jax.errors.JaxRuntimeError and friends
            if attempt == 2:
                raise
            import time
            time.sleep(2.0)
    out = np.empty((T, B * N), dtype=np.float32)
    for c in range(NCORES):
        zc = np.asarray(res.results[c]["y"])          # [P, T, F] uint8, z = 1-s
        r = zc.transpose(1, 0, 2).reshape(T, PERCORE)
        out[:, c * PERCORE:(c + 1) * PERCORE] = 1 - r
    return out.reshape(T, B, N), res


def kernel(x_seq):
    out, _ = run_sharded(x_seq)
    return out
